# revision 4
# baseline (speedup 1.0000x reference)
"""DeformMCALayer Trainium2 kernel: 8-way data-parallel over batch.

kernel(**inputs) takes the FULL inputs (x [8,256,64,64], offset_w [18,256,3,3],
deform_w [256,256,3,3]) and returns the FULL output [8,256,64,64] (float32).

Per core (one image):
  1. offset conv (3x3, f32r matmuls, PSUM-accumulated)
  2. PE-transpose offsets to pixel-partition layout; DVE index/bilinear-weight
     math (floor via 1.5*2^23 magic, clip, validity masks)
  3. x transposed to pixel-major bf16 in DRAM; SWDGE dma_gather fetches
     2-row pairs per (tap, pixel) -> pixel-per-partition tiles
  4. bilinear weighting: 4x-mode tensor_scalar muls + 2x tensor_tensor adds
  5. PE-transpose patches to (c,k)-partition tiles; bf16 matmul vs deform_w
     (contraction 2304) accumulating f32 in PSUM
  6. channel attention (mean + unbiased std -> sigmoid) fused on ACT reading
     PSUM; final scale + store
"""
import sys
sys.path.insert(0, "/opt/trn_rl_repo")
import numpy as np
import ml_dtypes

import concourse.bacc as bacc
import concourse.mybir as mybir
from concourse.tile import TileContext
from concourse.ap import AP

F32 = mybir.dt.float32
BF16 = mybir.dt.bfloat16
I16 = mybir.dt.int16
OP = mybir.AluOpType
AF = mybir.ActivationFunctionType

H = W = 64
HW = 4096
K = 9
NQT = 4
QPIX = 1024
XTROWS = 4104
N_CORES = 8
MAGIC = float(3 * 2 ** 22)  # 1.5*2^23 round-to-int magic (|x| < 2^22)


def _mk(ap_or_handle, extra_offset, dims):
    if isinstance(ap_or_handle, AP):
        t, off = ap_or_handle.tensor, ap_or_handle.offset
    else:
        a = ap_or_handle.ap()
        t, off = a.tensor, a.offset
    return AP(t, off + extra_offset, [list(d) for d in dims])


def build_program():
    nc = bacc.Bacc("TRN2", target_bir_lowering=False, debug=False, num_devices=1)

    x_d = nc.dram_tensor("x", [256, HW], F32, kind="ExternalInput")
    offw_d = nc.dram_tensor("offw", [128, 2, K, 18], mybir.dt.float32r, kind="ExternalInput")
    w2_d = nc.dram_tensor("w2", [128, 18, 256], BF16, kind="ExternalInput")
    basey_d = nc.dram_tensor("basey", [128, 32, K], F32, kind="ExternalInput")
    basex_d = nc.dram_tensor("basex", [128, 32, K], F32, kind="ExternalInput")
    idf_d = nc.dram_tensor("idf", [128, 128], F32, kind="ExternalInput")
    idb_d = nc.dram_tensor("idb", [128, 128], BF16, kind="ExternalInput")

    xt_d = nc.dram_tensor("xt", [XTROWS, 256], BF16, kind="Internal")
    y_d = nc.dram_tensor("y", [256, HW], F32, kind="ExternalOutput")

    with TileContext(nc) as tc:
        with tc.tile_pool(name="const", bufs=1) as cpool:
            w2_sb = cpool.tile([128, 18, 256], BF16)
            nc.sync.dma_start(w2_sb[:], w2_d[:])
            offw_sb = cpool.tile([128, 2, K, 18], mybir.dt.float32r)
            nc.sync.dma_start(offw_sb[:], offw_d[:])
            basey_sb = cpool.tile([128, 32, K], F32)
            nc.sync.dma_start(basey_sb[:], basey_d[:])
            basex_sb = cpool.tile([128, 32, K], F32)
            nc.sync.dma_start(basex_sb[:], basex_d[:])
            idf_sb = cpool.tile([128, 128], F32)
            nc.sync.dma_start(idf_sb[:], idf_d[:])
            idb_sb = cpool.tile([128, 128], BF16)
            nc.sync.dma_start(idb_sb[:], idb_d[:])
            zrow = cpool.tile([1, 2048], BF16)
            nc.vector.memset(zrow[:], 0.0)
            nc.sync.dma_start(_mk(xt_d, 4096 * 256, [[256, 8], [1, 256]]), zrow[:])

            offT = cpool.tile([128, 32, 18], F32)
            w4 = cpool.tile([128, 32, K, 4], F32)
            idxw16 = cpool.tile([16, K, 2, NQT, 64], I16)
            idxw = cpool.tile([128, K, 2, NQT, 64], I16)

            # ---------------- boot: conv + transposes ----------------
            with tc.tile_pool(name="boot", bufs=1) as bpool, \
                 tc.tile_pool(name="psconv", bufs=2, space="PSUM") as psconv, \
                 tc.tile_pool(name="pstp", bufs=2, space="PSUM") as pstp:
                x_pad = bpool.tile([128, 2, 66, 66], mybir.dt.float32r)
                nc.vector.memset(x_pad[:, :, 0, :].bitcast(F32), 0.0)
                nc.vector.memset(x_pad[:, :, 65, :].bitcast(F32), 0.0)
                nc.vector.memset(x_pad[:, :, 1:65, 0:1].bitcast(F32), 0.0)
                nc.vector.memset(x_pad[:, :, 1:65, 65:66].bitcast(F32), 0.0)
                for cb in range(2):
                    src = _mk(x_d, cb * 128 * HW, [[HW, 128], [64, 64], [1, 64]])
                    nc.gpsimd.dma_start(x_pad[:, cb, 1:65, 1:65], src)

                off_sb = bpool.tile([18, HW], F32)
                for chk in range(8):
                    ps_conv = psconv.tile([18, 512], F32, tag="conv")
                    r0 = chk * 8
                    idx = 0
                    for cb in range(2):
                        for k in range(K):
                            ky, kx = k // 3, k % 3
                            rhs = x_pad[:, cb, r0 + ky: r0 + ky + 8, kx: kx + 64]
                            nc.tensor.matmul(
                                ps_conv[:], offw_sb[:, cb, k, :], rhs,
                                start=(idx == 0), stop=(idx == 17))
                            idx += 1
                    nc.scalar.copy(off_sb[:, chk * 512:(chk + 1) * 512], ps_conv[:])

                for b in range(32):
                    ps_t = pstp.tile([128, 18], F32, tag="tp18")
                    nc.tensor.transpose(ps_t[:], off_sb[:, b * 128:(b + 1) * 128],
                                        idf_sb[0:18, 0:18])
                    nc.scalar.copy(offT[:, b, :], ps_t[:])

                x_unpad = bpool.tile([128, 2, HW], F32)
                for cb in range(2):
                    nc.sync.dma_start(
                        x_unpad[:, cb, :],
                        _mk(x_d, cb * 128 * HW, [[HW, 128], [1, HW]]))
                xT_sb = bpool.tile([128, 32, 256], BF16)
                for cb in range(2):
                    for b in range(32):
                        ps_x = pstp.tile([128, 128], F32, tag="tpx")
                        in_ap = x_unpad[:, cb, b * 128:(b + 1) * 128]
                        nc.tensor.transpose(ps_x[:], in_ap, idf_sb[:])
                        nc.scalar.copy(xT_sb[:, b, cb * 128:(cb + 1) * 128], ps_x[:])
                dst = _mk(xt_d, 0, [[256, 128], [128 * 256, 32], [1, 256]])
                nc.sync.dma_start(dst, xT_sb[:])

            # ---------------- index & weight math ----------------
            with tc.tile_pool(name="idx", bufs=1) as ipool, \
                 tc.tile_pool(name="psw", bufs=2, space="PSUM") as psw:
                def it(name):
                    return ipool.tile([128, 32, K], F32, tag=name, name=name)

                sy = it("sy"); sx = it("sx")
                nc.vector.tensor_add(sy[:], basey_sb[:], offT[:, :, 0:K])
                nc.vector.tensor_add(sx[:], basex_sb[:], offT[:, :, K:18])

                def floor_(s_t, name):
                    t = it(name + "_t"); c = it(name + "_c")
                    f = it(name + "_f"); l = it(name + "_l")
                    nc.vector.tensor_scalar_add(t[:], s_t[:], MAGIC)
                    nc.vector.tensor_scalar_sub(t[:], t[:], MAGIC)
                    nc.vector.tensor_tensor(c[:], t[:], s_t[:], OP.is_gt)
                    nc.vector.tensor_sub(f[:], t[:], c[:])
                    nc.vector.tensor_sub(l[:], s_t[:], f[:])
                    return f, l

                y0, ly = floor_(sy, "y")
                x0, lx = floor_(sx, "x")

                yc0 = it("yc0"); yc1 = it("yc1"); xc0 = it("xc0"); y1 = it("y1")
                nc.vector.tensor_scalar(yc0[:], y0[:], 0.0, 63.0, OP.max, OP.min)
                nc.vector.tensor_scalar_add(y1[:], y0[:], 1.0)
                nc.vector.tensor_scalar(yc1[:], y1[:], 0.0, 63.0, OP.max, OP.min)
                nc.vector.tensor_scalar(xc0[:], x0[:], 0.0, 63.0, OP.max, OP.min)

                def vmask(src_t, lo, hi, name):
                    a = it(name + "_a"); b = it(name + "_b"); v = it(name + "_v")
                    nc.vector.tensor_scalar(a[:], src_t[:], float(lo), None, OP.is_ge)
                    nc.vector.tensor_scalar(b[:], src_t[:], float(hi), None, OP.is_le)
                    nc.vector.tensor_mul(v[:], a[:], b[:])
                    return v

                vy0 = vmask(y0, 0, 63, "vy0")
                vy1 = vmask(y1, 0, 63, "vy1")
                vx0 = vmask(x0, 0, 63, "vx0")
                cx62 = vmask(x0, 0, 62, "cx62")
                ex = it("ex")
                nc.vector.tensor_scalar(ex[:], x0[:], -1.0, None, OP.is_equal)

                oly = it("oly"); olx = it("olx")
                nc.vector.tensor_scalar(oly[:], ly[:], -1.0, 1.0, OP.mult, OP.add)
                nc.vector.tensor_scalar(olx[:], lx[:], -1.0, 1.0, OP.mult, OP.add)

                wy0 = it("wy0"); wy1 = it("wy1"); wx0 = it("wx0"); wx1 = it("wx1")
                nc.vector.tensor_mul(wy0[:], oly[:], vy0[:])
                nc.vector.tensor_mul(wy1[:], ly[:], vy1[:])
                t1 = it("t1"); t2 = it("t2")
                nc.vector.tensor_mul(t1[:], olx[:], vx0[:])
                nc.vector.tensor_mul(t2[:], lx[:], ex[:])
                nc.vector.tensor_add(wx0[:], t1[:], t2[:])
                nc.vector.tensor_mul(wx1[:], lx[:], cx62[:])

                for s, (a_t, b_t) in enumerate([(wy0, wx0), (wy0, wx1),
                                                (wy1, wx0), (wy1, wx1)]):
                    nc.vector.tensor_tensor(w4[:, :, :, s], a_t[:], b_t[:], OP.mult)

                ida = it("ida"); idb_t = it("idb"); m1 = it("m1"); m2 = it("m2")
                nc.vector.tensor_scalar_mul(m1[:], yc0[:], 64.0)
                nc.vector.tensor_add(ida[:], m1[:], xc0[:])
                nc.vector.tensor_scalar_mul(m2[:], yc1[:], 64.0)
                nc.vector.tensor_add(idb_t[:], m2[:], xc0[:])

                idxf = ipool.tile([128, K, 2, 32], F32, tag="idxf")
                for ab, src_t in enumerate([ida, idb_t]):
                    src_ap = _mk(src_t[:], 0, [list(src_t[:].ap[0]), [1, K], [K, 32]])
                    dst_ap = _mk(idxf[:], ab * 32, [list(idxf[:].ap[0]), [64, K], [1, 32]])
                    nc.vector.tensor_copy(dst_ap, src_ap)

                T1_sb = ipool.tile([128, 5, 128], F32, tag="T1")
                nc.vector.memset(T1_sb[:], 0.0)
                widths = [128, 128, 128, 128, 64]
                for ch in range(5):
                    wd = widths[ch]
                    ps = psw.tile([128, 128], F32, tag="tpw")
                    in_ap = _mk(idxf[:], ch * 128, [list(idxf[:].ap[0]), [1, wd]])
                    nc.tensor.transpose(ps[0:wd, :], in_ap, idf_sb[:])
                    nc.scalar.copy(T1_sb[0:wd, ch, :], ps[0:wd, :])
                nc.vector.memset(idxw16[:], 0)
                for q in range(8):
                    for ch in range(5):
                        wd = widths[ch]
                        ps2 = psw.tile([16, 128], F32, tag="tpw2")
                        in2 = T1_sb[:, ch, q * 16: q * 16 + 16]
                        nc.tensor.transpose(ps2[:], in2, idf_sb[:])
                        base = idxw16[:].offset + (ch * 2) * 512 + q
                        pa = list(idxw16[:].ap[0])
                        pa[1] = 16
                        if wd == 128:
                            dims = [pa, [512, 2], [256, 2], [64, 4], [8, 8]]
                        else:
                            dims = [pa, [256, 2], [64, 4], [8, 8]]
                        dst_ap = AP(idxw16[:].tensor, base, dims)
                        nc.vector.tensor_copy(dst_ap, ps2[:, 0:wd])
                for cgrp in range(8):
                    nc.sync.dma_start(idxw[cgrp * 16:(cgrp + 1) * 16], idxw16[:])

            # ---------------- main: gather, weight, transpose, matmul ----------
            with tc.tile_pool(name="main", bufs=2) as mpool, \
                 tc.tile_pool(name="ybuf", bufs=1) as ypool, \
                 tc.tile_pool(name="pstpp", bufs=4, space="PSUM") as pstpp, \
                 tc.tile_pool(name="psmm", bufs=3, space="PSUM") as psmm:
                y_sb = ypool.tile([128, 2, HW], F32)
                s1p = ypool.tile([128, 2, 8], F32)
                s2p = ypool.tile([128, 2, 8], F32)
                gsrc_ap = _mk(xt_d, 0, [[256, XTROWS - 1], [1, 512]])

                for qt in range(NQT):
                    patchT = mpool.tile([128, 18, QPIX], BF16, tag="patchT")
                    for k in range(K):
                        ga = mpool.tile([128, 8, 512], BF16, tag="ga")
                        gb = mpool.tile([128, 8, 512], BF16, tag="gb")
                        nc.gpsimd.dma_gather(ga[:], gsrc_ap, idxw[:, k, 0, qt, :],
                                             QPIX, QPIX, 512, elem_step=256)
                        nc.gpsimd.dma_gather(gb[:], gsrc_ap, idxw[:, k, 1, qt, :],
                                             QPIX, QPIX, 512, elem_step=256)
                        a0 = mpool.tile([128, 8, 256], BF16, tag="wa0", bufs=1)
                        a1 = mpool.tile([128, 8, 256], BF16, tag="wa1", bufs=1)
                        a2 = mpool.tile([128, 8, 256], BF16, tag="wa2", bufs=1)
                        a3 = mpool.tile([128, 8, 256], BF16, tag="wa3", bufs=1)
                        p_t = mpool.tile([128, 8, 256], BF16, tag="tp")
                        for g in range(8):
                            blk = qt * 8 + g
                            srcs = (ga[:, g, 0:256], ga[:, g, 256:512],
                                    gb[:, g, 0:256], gb[:, g, 256:512])
                            for sl, (dst_t, src) in enumerate(
                                    zip((a0, a1, a2, a3), srcs)):
                                nc.vector.tensor_scalar(
                                    dst_t[:, g, :], src,
                                    w4[:, blk, k, sl:sl + 1], None, OP.mult)
                        nc.vector.tensor_add(a0[:], a0[:], a1[:])
                        nc.vector.tensor_add(a2[:], a2[:], a3[:])
                        nc.vector.tensor_add(p_t[:], a0[:], a2[:])
                        for cb in range(2):
                            for gh in range(2):
                                psx = pstpp.tile([128, 4, 128], BF16, tag="tpp")
                                for gi in range(4):
                                    g = gh * 4 + gi
                                    nc.tensor.transpose(
                                        psx[:, gi, :],
                                        p_t[:, g, cb * 128:(cb + 1) * 128],
                                        idb_sb[:])
                                dst = patchT[:, k * 2 + cb,
                                             gh * 512:(gh + 1) * 512]
                                if (cb + gh + k) % 3 == 2:
                                    nc.vector.tensor_copy(dst, psx[:])
                                else:
                                    nc.scalar.copy(dst, psx[:])
                    for chunk in range(2):
                        for oh in range(2):
                            psd = psmm.tile([128, 512], F32, tag="mm")
                            for kc in range(18):
                                nc.tensor.matmul(
                                    psd[:], w2_sb[:, kc, oh * 128:(oh + 1) * 128],
                                    patchT[:, kc, chunk * 512:(chunk + 1) * 512],
                                    start=(kc == 0), stop=(kc == 17))
                            cidx = qt * 2 + chunk
                            nc.scalar.activation(
                                y_sb[:, oh, qt * 1024 + chunk * 512:
                                     qt * 1024 + (chunk + 1) * 512],
                                psd[:], AF.Copy, accum_out=s1p[:, oh, cidx:cidx + 1])
                            sqscr = mpool.tile([128, 512], BF16, tag="sq")
                            nc.scalar.activation(
                                sqscr[:], psd[:], AF.Square,
                                accum_out=s2p[:, oh, cidx:cidx + 1])

                # ---------------- stats + scale ----------------
                s1 = ypool.tile([128, 2], F32)
                s2 = ypool.tile([128, 2], F32)
                nc.vector.reduce_sum(s1[:], s1p[:], axis=mybir.AxisListType.X)
                nc.vector.reduce_sum(s2[:], s2p[:], axis=mybir.AxisListType.X)
                mean = ypool.tile([128, 2], F32)
                nc.vector.tensor_scalar_mul(mean[:], s1[:], 1.0 / HW)
                ss = ypool.tile([128, 2], F32)
                nc.vector.tensor_mul(ss[:], s1[:], s1[:])
                va = ypool.tile([128, 2], F32)
                vb = ypool.tile([128, 2], F32)
                var = ypool.tile([128, 2], F32)
                nc.vector.tensor_scalar_mul(va[:], s2[:], 1.0 / (HW - 1))
                nc.vector.tensor_scalar_mul(vb[:], ss[:], 1.0 / (HW * (HW - 1.0)))
                nc.vector.tensor_sub(var[:], va[:], vb[:])
                nc.vector.tensor_scalar_max(var[:], var[:], 0.0)
                std = ypool.tile([128, 2], F32)
                nc.scalar.sqrt(std[:], var[:])
                arg = ypool.tile([128, 2], F32)
                nc.vector.tensor_add(arg[:], mean[:], std[:])
                attn = ypool.tile([128, 2], F32)
                nc.scalar.activation(attn[:], arg[:], AF.Sigmoid)
                for oh in range(2):
                    nc.vector.tensor_scalar_mul(y_sb[:, oh, :], y_sb[:, oh, :],
                                                attn[:, oh:oh + 1])
                    nc.sync.dma_start(
                        _mk(y_d, oh * 128 * HW, [[HW, 128], [1, HW]]),
                        y_sb[:, oh, :])

    nc.compile()
    return nc


def _prep_shared(offset_w, deform_w):
    perm = [2 * i for i in range(9)] + [2 * i + 1 for i in range(9)]
    wp = np.asarray(offset_w, np.float32)[perm]
    wp2 = wp.reshape(18, 2, 128, 9)
    offw = np.ascontiguousarray(wp2.transpose(2, 1, 3, 0)).astype(np.float32)

    wk = np.asarray(deform_w, np.float32).reshape(256, 256, 9)
    t = wk.reshape(256, 2, 128, 9).transpose(2, 3, 1, 0)
    w2 = np.ascontiguousarray(t.reshape(128, 18, 256)).astype(ml_dtypes.bfloat16)

    p = np.arange(128)
    blk = np.arange(32)
    kk = np.arange(9)
    i_pix = blk[None, :, None] * 2 + (p[:, None, None] // 64)
    j_pix = (p[:, None, None] % 64) + 0 * blk[None, :, None]
    basey = np.ascontiguousarray(np.broadcast_to(
        (i_pix + (kk // 3)[None, None, :] - 1), (128, 32, 9))).astype(np.float32)
    basex = np.ascontiguousarray(np.broadcast_to(
        (j_pix + (kk % 3)[None, None, :] - 1), (128, 32, 9))).astype(np.float32)

    idf = np.eye(128, dtype=np.float32)
    idb = np.eye(128, dtype=np.float32).astype(ml_dtypes.bfloat16)
    return dict(offw=offw, w2=w2, basey=basey, basex=basex, idf=idf, idb=idb)


_CACHE = {}


def kernel(x, offset_w, deform_w):
    x = np.asarray(x, np.float32)
    B = x.shape[0]
    assert x.shape == (8, 256, 64, 64)

    if "nc" not in _CACHE:
        _CACHE["nc"] = build_program()
    nc = _CACHE["nc"]

    shared = _prep_shared(offset_w, deform_w)
    in_maps = []
    for b in range(B):
        m = dict(shared)
        m["x"] = np.ascontiguousarray(x[b].reshape(256, HW))
        in_maps.append(m)

    from concourse.bass_utils import run_bass_kernel_spmd
    res = run_bass_kernel_spmd(nc, in_maps, core_ids=list(range(N_CORES)))
    out = np.stack([res.results[b]["y"].reshape(256, 64, 64) for b in range(B)])
    return out.astype(np.float32)


if __name__ == "__main__":
    d = np.load("/root/problem/ref_cache.npz")
    out = kernel(d["x"], d["offset_w"], d["deform_w"])
    err = np.abs(out - d["expected"]).max() / np.abs(d["expected"]).max()
    print("rel err vs cached ref:", err)
